# revision 12
# baseline (speedup 1.0000x reference)
"""Coordinate-descent (alternating Gauss-Seidel) kernel for Trainium2, v7.

B=4 factorizations x ~ u @ v^T, M=N=4096, R=32, row-sharded over 8 cores.

Design (v7):
 - All-f32r datapath (f32r = bit-reinterpret of f32; PE streams it at
   1 col/cycle like bf16, measured: 512-col MM 232ns vs 216ns bf16).
   No cast pass, no bf16 staging.
 - Solves collapsed to one MM each: ghat = (M^{-1}D'^{-1})^T precomputed
   per Gram via the nilpotent W-power chain.
 - v_full is loaded in a 128-descriptor contiguous layout (partition p
   holds rows p*32..p*32+31); phase-1 transposes slice x with a matching
   n-comb (n = sg*1024 + q*8 + jj) so the a1 stationary v slices align.
   DMA issue cost is ~4ns/descriptor on the issuing engine, so the old
   4096-descriptor v loads were burning ~16us of queue time each.
 - x for batch b>=1 is fully prefetched during stream b-1 (2 DMAs, 512
   descriptors each); only batch 0 streams group-wise.
 - f32 collectives (bf16 RS measured err 1.5e-2, too close to the gate).
 - RS-result loads ride the gpsimd queue, serialized behind the
   collective itself, so no compute queue ever blocks on the RS.
 - u-solve epilogue of batch b runs at the start of stream b+1; the
   v-solve of b-1 is interleaved with the a1 flush at stream b's end.
"""

import os
from contextlib import ExitStack

import numpy as np

import concourse.bass as bass
import concourse.tile as tile
from concourse import bacc, mybir
from concourse.bass import ds
from concourse.bass_utils import run_bass_kernel_spmd
from concourse.masks import make_identity, make_lower_triangular

B, M, N, R = 4, 4096, 4096, 32
NCORES = 8
MS = M // NCORES          # 512 rows per core per batch
MC = MS // 128            # 4 m-chunks of 128
NG = N // 512             # 8 n-groups of 512
NSG = 4                   # super-groups of 1024 cols (comb granularity)
NCH = N // 128            # 32 n-chunks of 128
EPS = 1e-8
F32 = mybir.dt.float32
F32R = mybir.dt.float32r
BF16 = mybir.dt.bfloat16
ALU = mybir.AluOpType
AX = mybir.AxisListType

_CACHE = {}
LAST_RESULT = None


def _gram_prep(nc, smp, pwp, punp, consts, b_sb, tg):
    """From Gram b precompute nbsl and ghat = (M^{-1}D'^{-1})^T.

    M^T = D' + bsl (b symmetric), W = D'^{-1} bsl strictly lower,
    (I+W)^{-1} = (I-W)(I+W^2)(I+W^4)(I+W^8)(I+W^16) exactly (W^32 = 0).
    ghat = D'^{-1} G with G = [(I+W)^{-1}]^T; then
    u_new^T = ghat.T @ (a^T - bsl^T u^T + eps).
    """
    ident32_r, masksl_r, eye_r = consts

    bd = smp.tile([R, R], F32R, tag=f"bd{tg}", name="bd", bufs=1)
    nc.vector.tensor_tensor(out=bd[:], in0=b_sb, in1=eye_r, op=ALU.mult)
    d_p = smp.tile([R, 1], F32, tag=f"dp{tg}", name="d_p", bufs=1)
    inv_p = smp.tile([R, 1], F32, tag=f"ip{tg}", name="inv_p", bufs=1)
    nc.vector.tensor_reduce(d_p[:], bd[:], axis=AX.X, op=ALU.add)
    nc.vector.tensor_scalar_add(inv_p[:], d_p[:], EPS)
    nc.vector.reciprocal(inv_p[:], inv_p[:])
    invb = bass.AP(inv_p[:].tensor, inv_p[:].offset, [inv_p[:].ap[0], [0, R]])

    bsl = smp.tile([R, R], F32R, tag=f"bsl{tg}", name="bsl", bufs=1)
    nc.vector.tensor_tensor(out=bsl[:], in0=b_sb, in1=masksl_r,
                            op=ALU.mult)
    nbsl = smp.tile([R, R], F32R, tag=f"nbsl{tg}", name="nbsl", bufs=1)
    nc.vector.tensor_scalar_mul(nbsl[:], bsl[:], -1.0)
    vw = smp.tile([R, R], F32R, tag=f"vw{tg}", name="vw", bufs=1)  # W
    nc.vector.tensor_tensor(out=vw[:], in0=bsl[:], in1=invb, op=ALU.mult)

    # transpose W -> W^T
    pwt = punp.tile([128, MC, R], F32R, tag="pun", name="pwt")
    nc.tensor.transpose(pwt[:R, 0, :], vw[:], ident32_r)
    w1 = smp.tile([R, R], F32R, tag=f"w1{tg}", name="w1", bufs=1)
    nc.scalar.copy(w1[:], pwt[:R, 0, :])

    def _mm_small(lhsT, rhs, tagn):
        p = pwp.tile([R, R], F32, tag="pw", name="pmm")
        nc.tensor.matmul(p[:], lhsT=lhsT[:], rhs=rhs[:], start=True,
                         stop=True)
        s = smp.tile([R, R], F32R, tag=f"{tagn}{tg}", name=tagn, bufs=1)
        nc.scalar.copy(s[:], p[:])
        return s

    # powers of W: plain tiles are (W^k)^T, "t" tiles are W^k
    w2 = _mm_small(vw, w1, "w2")      # (W^2)^T
    w2t = _mm_small(w1, vw, "w2t")    # W^2
    w4 = _mm_small(w2t, w2, "w4")     # (W^4)^T
    w4t = _mm_small(w2, w2t, "w4t")   # W^4
    w8 = _mm_small(w4t, w4, "w8")     # (W^8)^T
    w8t = _mm_small(w4, w4t, "w8t")   # W^8
    w16t = _mm_small(w8, w8t, "w16t")  # W^16

    # G chain: G0 = I - W^T; G <- G + (W^{2^k})^T G  (lhsT = W^{2^k})
    g = smp.tile([R, R], F32R, tag=f"g0{tg}", name="g0", bufs=1)
    nc.vector.tensor_tensor(out=g[:], in0=ident32_r, in1=w1[:],
                            op=ALU.subtract)
    for k, wkt in enumerate((w2t, w4t, w8t, w16t)):
        pg = pwp.tile([R, R], F32, tag="pw", name="pg")
        nc.tensor.matmul(pg[:], lhsT=wkt[:], rhs=g[:], start=True, stop=True)
        gn = smp.tile([R, R], F32R, tag=f"g{k + 1}{tg}", name="gn", bufs=1)
        nc.vector.tensor_tensor(out=gn[:], in0=g[:], in1=pg[:], op=ALU.add)
        g = gn
    ghat = smp.tile([R, R], F32R, tag=f"gh{tg}", name="ghat", bufs=1)
    nc.vector.tensor_tensor(out=ghat[:], in0=g[:], in1=invb, op=ALU.mult)

    return {"nbsl": nbsl, "ghat": ghat}


def _build():
    nc = bacc.Bacc("TRN2", target_bir_lowering=False, debug=False,
                   num_devices=NCORES)

    x_my = nc.dram_tensor("x_my", [B, MS, N], F32, kind="ExternalInput").ap()
    u_my = nc.dram_tensor("u_my", [B, MS, R], F32, kind="ExternalInput").ap()
    v_full = nc.dram_tensor("v_full", [B, N, R], F32,
                            kind="ExternalInput").ap()
    v_my = nc.dram_tensor("v_my", [B, MS, R], F32, kind="ExternalInput").ap()
    u_out = nc.dram_tensor("u_out", [B, MS, R], F32,
                           kind="ExternalOutput").ap()
    v_out = nc.dram_tensor("v_out", [B, MS, R], F32,
                           kind="ExternalOutput").ap()

    # f32 collective payloads; batch 3's RS is split into two halves:
    # half a = a2T cols 0:256 + b2, half b = a2T cols 256:512.
    rs_ins = [nc.dram_tensor(f"rs_in_{b}", [NCORES * R, 512 + R], F32)
              for b in range(B - 1)]
    rs_outs = [nc.dram_tensor(f"rs_out_{b}", [R, 512 + R], F32)
               for b in range(B - 1)]
    rs_in_3a = nc.dram_tensor("rs_in_3a", [NCORES * R, 256 + R], F32)
    rs_out_3a = nc.dram_tensor("rs_out_3a", [R, 256 + R], F32)
    rs_in_3b = nc.dram_tensor("rs_in_3b", [NCORES * R, 256], F32)
    rs_out_3b = nc.dram_tensor("rs_out_3b", [R, 256], F32)

    with tile.TileContext(nc) as tc, ExitStack() as ctx:
        const = ctx.enter_context(tc.tile_pool(name="const", bufs=1))
        xbp = ctx.enter_context(tc.tile_pool(name="xbp", bufs=1))
        xtp = ctx.enter_context(tc.tile_pool(name="xtp", bufs=8))
        vp = ctx.enter_context(tc.tile_pool(name="vp", bufs=2))
        smp = ctx.enter_context(tc.tile_pool(name="smp", bufs=2))
        a2sp = ctx.enter_context(tc.tile_pool(name="a2sp", bufs=2))
        # PSUM banks: ppt 2 + pa1 1 + pa2 1 + pw 1 + pun 1 + psol 2 = 8
        ppt = ctx.enter_context(tc.tile_pool(name="ppt", bufs=2,
                                             space="PSUM"))
        pa1p = ctx.enter_context(tc.tile_pool(name="pa1", bufs=1,
                                              space="PSUM"))
        pa2p = ctx.enter_context(tc.tile_pool(name="pa2", bufs=1,
                                              space="PSUM"))
        pwp = ctx.enter_context(tc.tile_pool(name="pw", bufs=1,
                                             space="PSUM"))
        punp = ctx.enter_context(tc.tile_pool(name="pun", bufs=1,
                                              space="PSUM"))
        psolp = ctx.enter_context(tc.tile_pool(name="psol", bufs=2,
                                               space="PSUM"))

        ident128_f = const.tile([128, 128], F32)
        make_identity(nc, ident128_f)
        ident128_r = const.tile([128, 128], F32R)
        nc.vector.tensor_copy(ident128_r[:], ident128_f[:])
        ident32_f = const.tile([R, R], F32)
        make_identity(nc, ident32_f)
        masksl_f = const.tile([R, R], F32)
        make_lower_triangular(nc, masksl_f, val=1.0, diag=False)
        ident32_r = const.tile([R, R], F32R)
        nc.vector.tensor_copy(ident32_r[:], ident32_f[:])
        masksl_r = const.tile([R, R], F32R)
        nc.vector.tensor_copy(masksl_r[:], masksl_f[:])
        consts = (ident32_r[:], masksl_r[:], ident32_r[:])

        # ---------- x(b0) sg0 on sync first, then fast v loads on scalar --
        xbs = {}

        def xb_tile(b):
            t = xbp.tile([128, MC, N], F32R, tag=f"xb{b % 2}", name="xb")
            xbs[b] = t
            return t

        def x_dram(b):
            return x_my[b].rearrange("(i p) n -> p i n", p=128).bitcast(F32R)

        xb0 = xb_tile(0)
        nc.sync.dma_start(xb0[:, :, 0:1024], x_dram(0)[:, :, 0:1024])
        nc.sync.dma_start(xb0[:, :, 1024:2048], x_dram(0)[:, :, 1024:2048])

        # v_full in comb layout: partition p holds n = sg*1024 + p*8 + jj
        # at index (sg*8 + jj); 512 descriptors of 1KB each.
        vts = []
        for b in range(B):
            v32 = vp.tile([128, NCH, R], F32R, tag=f"v32{b}", name="v32",
                          bufs=1)
            nc.scalar.dma_start(
                v32.rearrange("p (sg jj) r -> p sg jj r", sg=NSG),
                v_full[b].rearrange("(sg p jj) r -> p sg jj r",
                                    p=128, jj=8).bitcast(F32R))
            vts.append(v32)

        u_preps = {}

        def emit_prep(b):
            pb1 = pwp.tile([R, R], F32, tag="pw", name="pb1")
            for j in range(NCH):
                nc.tensor.matmul(pb1[:], lhsT=vts[b][:, j, :],
                                 rhs=vts[b][:, j, :], start=(j == 0),
                                 stop=(j == NCH - 1), skip_group_check=True)
            b1_sb = smp.tile([R, R], F32R, tag=f"b1s{b}", name="b1_sb",
                             bufs=1)
            nc.scalar.copy(b1_sb[:], pb1[:])
            u_preps[b] = _gram_prep(nc, smp, pwp, punp, consts, b1_sb[:],
                                    f"u{b}")

        state = {}

        def back_transpose(zout, cols, out32, tg):
            """zout [R, len(cols)*128] (SBUF) -> out32 chunks via PE."""
            pun = punp.tile([128, MC, R], F32R, tag="pun", name=f"pun{tg}")
            for k, i in enumerate(cols):
                nc.tensor.transpose(pun[:, i, :],
                                    zout[:, k * 128:(k + 1) * 128],
                                    ident32_r[:])
            for k, i in enumerate(cols):
                nc.scalar.copy(out32[:, i, :], pun[:, i, :])

        def usolve_finish(b):
            """Epilogue of batch b's u-solve (pa1 group already closed)."""
            pa1 = state[b]["pa1"]
            zin = smp.tile([R, MS], F32R, tag="zinu", name="zinu", bufs=1)
            nc.vector.tensor_scalar_add(zin[:], pa1[:], EPS)
            pzu = psolp.tile([R, MS], F32, tag="psol", name="pzu")
            nc.tensor.matmul(pzu[:], lhsT=u_preps[b]["ghat"][:], rhs=zin[:],
                             start=True, stop=True, skip_group_check=True)
            zout = smp.tile([R, MS], F32R, tag="zou", name="zou", bufs=1)
            nc.scalar.copy(zout[:], pzu[:])
            un32 = smp.tile([128, MC, R], F32R, tag="un32", name="un32",
                            bufs=1)
            back_transpose(zout, range(MC), un32, "u")
            nc.scalar.dma_start(
                u_out[b].rearrange("(i p) r -> p i r", p=128).bitcast(F32R),
                un32[:])
            state[b]["un32"] = un32

        def emit_phase2_part(bp, gp):
            un_p = state[bp]["un32"]
            xb_p = xbs[bp]
            pa2 = pa2p.tile([R, MS], F32, tag="pa2", name="pa2")
            for i in range(MC):
                nc.tensor.matmul(
                    pa2[:], lhsT=un_p[:, i, :],
                    rhs=xb_p[:, i, gp * 512:(gp + 1) * 512],
                    start=(i == 0), stop=(i == MC - 1),
                    skip_group_check=True)
            a2st = a2sp.tile([R, MS], F32, tag="a2st", name="a2st")
            nc.vector.tensor_copy(a2st[:], pa2[:])
            if bp < B - 1:
                nc.scalar.dma_start(
                    rs_ins[bp].ap()[ds(gp * R, R), 0:512], a2st[:])
            else:
                nc.scalar.dma_start(
                    rs_in_3a.ap()[ds(gp * R, R), 0:256], a2st[:, 0:256])
                nc.scalar.dma_start(
                    rs_in_3b.ap()[ds(gp * R, R), 0:256], a2st[:, 256:512])

        def bcast_b2(b2st, rs_dram, col0):
            # one DMA writing all 8 replicas of b2: out iterates (r, c, k)
            out_ap = rs_dram.ap()[:, col0:col0 + R].rearrange(
                "(c r) k -> r c k", c=NCORES)
            src = b2st[:]
            in_ap = bass.AP(src.tensor, src.offset,
                            [src.ap[0], [0, NCORES], src.ap[1]])
            nc.scalar.dma_start(out_ap, in_ap)

        def emit_b2(bp):
            un_p = state[bp]["un32"]
            pb2 = pwp.tile([R, R], F32, tag="pw", name="pb2")
            for i in range(MC):
                nc.tensor.matmul(pb2[:], lhsT=un_p[:, i, :],
                                 rhs=un_p[:, i, :], start=(i == 0),
                                 stop=(i == MC - 1), skip_group_check=True)
            b2st = a2sp.tile([R, R], F32, tag="b2st", name="b2st")
            nc.scalar.copy(b2st[:], pb2[:])
            if bp < B - 1:
                bcast_b2(b2st, rs_ins[bp], 512)
            else:
                bcast_b2(b2st, rs_in_3a, 256)

        def emit_rs(bp):
            nc.gpsimd.collective_compute(
                "ReduceScatter", ALU.add,
                replica_groups=[list(range(NCORES))],
                ins=[rs_ins[bp].ap()], outs=[rs_outs[bp].ap()])

        def load_rs_result(bp):
            # gpsimd queue: serialized behind the collective itself
            st = state[bp]
            if bp < B - 1:
                a2t = smp.tile([R, 512 + R], F32, tag="a2t", name="a2t",
                               bufs=1)
                nc.gpsimd.dma_start(a2t[:], rs_outs[bp].ap())
                st["a2t_halves"] = [(a2t[:, 0:256], 0, 256),
                                    (a2t[:, 256:512], 256, 512)]
                st["b2"] = a2t[:, 512:512 + R]
            else:
                a2ta = smp.tile([R, 256 + R], F32, tag="a2t", name="a2ta",
                                bufs=1)
                nc.gpsimd.dma_start(a2ta[:], rs_out_3a.ap())
                st["a2t_halves"] = [(a2ta[:, 0:256], 0, 256), None]
                st["b2"] = a2ta[:, 256:256 + R]

        def load_rs_result_3b(bp):
            st = state[bp]
            a2tb = smp.tile([R, 256], F32, tag="a2tb", name="a2tb", bufs=1)
            nc.gpsimd.dma_start(a2tb[:], rs_out_3b.ap())
            st["a2t_halves"][1] = (a2tb[:], 256, 512)

        def emit_prep_v(b):
            st = state[b]
            st["vprep"] = _gram_prep(nc, smp, pwp, punp, consts, st["b2"],
                                     f"v{b}")

        def solve_v_ps(b):
            st = state[b]
            ps = psolp.tile([R, MS], F32, tag="psol", name="ps")
            nc.tensor.matmul(ps[:], lhsT=st["vprep"]["nbsl"][:],
                             rhs=st["vT"][:], start=True, stop=True,
                             skip_group_check=True)
            st["ps"] = ps

        def solve_v_zin(b, h):
            st = state[b]
            a2t, c0, c1 = st["a2t_halves"][h]
            zin = smp.tile([R, 256], F32R, tag=f"zinv{h}", name="zin",
                           bufs=1)
            nc.vector.scalar_tensor_tensor(
                out=zin[:], in0=st["ps"][:, c0:c1], scalar=EPS,
                in1=a2t, op0=ALU.add, op1=ALU.add)
            st[f"zin{h}"] = zin

        def solve_v_fin(b, h, vn32):
            st = state[b]
            pz = psolp.tile([R, 256], F32, tag="psol", name=f"pz{h}")
            nc.tensor.matmul(pz[:], lhsT=st["vprep"]["ghat"][:],
                             rhs=st[f"zin{h}"][:], start=True, stop=True,
                             skip_group_check=True)
            zov = smp.tile([R, 256], F32R, tag=f"zov{h}", name="zov",
                           bufs=1)
            nc.scalar.copy(zov[:], pz[:])
            back_transpose(zov, (2 * h, 2 * h + 1), vn32, f"v{h}")

        def vn_tile():
            return smp.tile([128, MC, R], F32R, tag="vn32", name="vn32",
                            bufs=1)

        def vout_dma(b, vn32):
            nc.scalar.dma_start(
                v_out[b].rearrange("(i p) r -> p i r", p=128).bitcast(F32R),
                vn32[:])

        # ================= main batch loop =================
        for b in range(B):
            xb = xbs[b]
            # comb view: n = sg*1024 + nn*8 + jj
            xbr = xb.rearrange("p i (sg nn jj) -> p i sg nn jj",
                               sg=NSG, jj=8)

            u32 = vp.tile([128, MC, R], F32R, tag="u32", name="u32")
            nc.scalar.dma_start(
                u32[:],
                u_my[b].rearrange("(i p) r -> p i r", p=128).bitcast(F32R))
            vm32 = vp.tile([128, MC, R], F32R, tag="vm32", name="vm32")
            nc.scalar.dma_start(
                vm32[:],
                v_my[b].rearrange("(i p) r -> p i r", p=128).bitcast(F32R))
            uvT = {}

            def emit_uvT():
                put = psolp.tile([R, MS], F32R, tag="psol", name="put")
                for i in range(MC):
                    nc.tensor.transpose(put[:, i * 128:(i + 1) * 128],
                                        u32[:, i, :], ident128_r[:])
                uT = smp.tile([R, MS], F32R, tag="uT", name="uT", bufs=1)
                nc.scalar.copy(uT[:], put[:])
                pvt = psolp.tile([R, MS], F32R, tag="psol", name="pvt")
                for i in range(MC):
                    nc.tensor.transpose(pvt[:, i * 128:(i + 1) * 128],
                                        vm32[:, i, :], ident128_r[:])
                vT = smp.tile([R, MS], F32R, tag="vT", name="vT", bufs=2)
                nc.scalar.copy(vT[:], pvt[:])
                uvT["uT"], uvT["vT"] = uT, vT

            # ---------------- phase 1: stream x ----------------
            pa1 = pa1p.tile([R, MS], F32, tag="pa1", name="pa1")

            def emit_a1(q):
                for j2 in range(4):
                    j = 4 * q + j2
                    xt = p1xt[j]
                    nc.tensor.matmul(pa1[:], lhsT=vts[b][:, j, :],
                                     rhs=xt.rearrange("p a b -> p (a b)"),
                                     start=(j == 0), stop=False,
                                     skip_group_check=True)

            p1xt = {}
            for q in range(NG):          # quartet q covers j = 4q..4q+3
                if b == 0 and q % 2 == 0 and q // 2 + 2 < NSG:
                    sg_n = q // 2 + 2
                    nc.sync.dma_start(
                        xb[:, :, sg_n * 1024:(sg_n + 1) * 1024],
                        x_dram(0)[:, :, sg_n * 1024:(sg_n + 1) * 1024])
                for j2 in range(4):
                    j = 4 * q + j2
                    sg, jj = j // 8, j % 8
                    pt = ppt.tile([128, MC, 128], F32R, tag="pt", name="pt")
                    for i in range(MC):
                        nc.tensor.transpose(pt[:, i],
                                            xbr[:, i, sg, :, jj],
                                            ident128_r[:])
                    xt = xtp.tile([128, MC, 128], F32R, tag="xt", name="xt")
                    nc.vector.tensor_copy(xt[:], pt[:])
                    p1xt[j] = xt
                if q >= 1:
                    emit_a1(q - 1)
                # slot work overlapped with the stream
                if b >= 1:
                    if q == 0:
                        usolve_finish(b - 1)
                        for gp in range(4):
                            emit_phase2_part(b - 1, gp)
                    elif q == 1:
                        for gp in range(4, 8):
                            emit_phase2_part(b - 1, gp)
                        emit_b2(b - 1)
                        emit_rs(b - 1)
                    elif q == 6:
                        load_rs_result(b - 1)
                    elif q == 7:
                        emit_prep_v(b - 1)
                if q == 2:
                    emit_uvT()
                if b == 0 and 1 <= q <= B:
                    emit_prep(q - 1)
                if q == 4 and b + 1 < B:
                    xb_n = xb_tile(b + 1)
                    nc.sync.dma_start(xb_n[:, :, 0:1024],
                                      x_dram(b + 1)[:, :, 0:1024])
                    nc.sync.dma_start(xb_n[:, :, 1024:4096],
                                      x_dram(b + 1)[:, :, 1024:4096])

            # ------------- stream end: a1 flush + v-solve(b-1) ----------
            uT, vT = uvT["uT"], uvT["vT"]
            if b >= 1:
                solve_v_ps(b - 1)
            emit_a1(NG - 1)
            if b >= 1:
                solve_v_zin(b - 1, 0)
                solve_v_zin(b - 1, 1)
            nc.tensor.matmul(pa1[:], lhsT=u_preps[b]["nbsl"][:], rhs=uT[:],
                             start=False, stop=True, skip_group_check=True)
            if b >= 1:
                vn32 = vn_tile()
                solve_v_fin(b - 1, 0, vn32)
                solve_v_fin(b - 1, 1, vn32)
                vout_dma(b - 1, vn32)

            state[b] = {"pa1": pa1, "uT": uT, "vT": vT}

        # ================= tail: batch 3 =================
        bl = B - 1
        usolve_finish(bl)
        emit_b2(bl)
        for gp in range(8):
            emit_phase2_part(bl, gp)
        nc.gpsimd.collective_compute(
            "ReduceScatter", ALU.add,
            replica_groups=[list(range(NCORES))],
            ins=[rs_in_3a.ap()], outs=[rs_out_3a.ap()])
        load_rs_result(bl)
        nc.gpsimd.collective_compute(
            "ReduceScatter", ALU.add,
            replica_groups=[list(range(NCORES))],
            ins=[rs_in_3b.ap()], outs=[rs_out_3b.ap()])
        load_rs_result_3b(bl)
        emit_prep_v(bl)
        solve_v_ps(bl)
        vn32 = vn_tile()
        solve_v_zin(bl, 0)
        solve_v_fin(bl, 0, vn32)
        solve_v_zin(bl, 1)
        solve_v_fin(bl, 1, vn32)
        vout_dma(bl, vn32)

    nc.compile()
    return nc


def kernel(x, u, v):
    global LAST_RESULT
    if "nc" not in _CACHE:
        _CACHE["nc"] = _build()
    nc = _CACHE["nc"]

    x = np.ascontiguousarray(x, dtype=np.float32)
    u = np.ascontiguousarray(u, dtype=np.float32)
    v = np.ascontiguousarray(v, dtype=np.float32)

    in_maps = []
    for c in range(NCORES):
        sl = slice(c * MS, (c + 1) * MS)
        in_maps.append({
            "x_my": np.ascontiguousarray(x[:, sl, :]),
            "u_my": np.ascontiguousarray(u[:, sl, :]),
            "v_full": v,
            "v_my": np.ascontiguousarray(v[:, sl, :]),
        })

    res = run_bass_kernel_spmd(nc, in_maps, list(range(NCORES)),
                               trace=os.environ.get("KBENCH_TRACE") == "1")
    LAST_RESULT = res
    u_new = np.concatenate([res.results[c]["u_out"] for c in range(NCORES)],
                           axis=1)
    v_new = np.concatenate([res.results[c]["v_out"] for c in range(NCORES)],
                           axis=1)
    return (u_new, v_new)


# revision 15
# speedup vs baseline: 1.4112x; 1.4112x over previous
"""Coordinate-descent (alternating Gauss-Seidel) kernel for Trainium2, v8.

B=4 factorizations x ~ u @ v^T, M=N=4096, R=32, row-sharded over 8 cores.

Design (v8):
 - All-f32r datapath (f32r = bit-reinterpret of f32; PE streams it at
   1 col/cycle like bf16).  No cast pass, no bf16 staging.
 - Solves collapsed to one MM each: ghat = (M^{-1}D'^{-1})^T precomputed
   per Gram via the nilpotent W-power chain.
 - Phase-1 transposes use consecutive 128-col chunks (contiguous
   stationary reads; strided comb LDWs measured ~2x slower).  v_full is
   loaded in chunk layout as 4 pieces alternated across the two HWDGE
   queues so no single queue eats the 16us descriptor storm.
 - x for batch b>=1 fully prefetched during stream b-1 (2 DMAs); batch 0
   streams 1024-col super-groups.
 - f32 collectives.  RS-result loads ride the gpsimd queue (serialized
   behind the collective, blocking no compute queue).
 - u-solve epilogue of batch b runs at stream b+1 q0; v-solve of batch b
   runs at stream b+2's end (a full stream of slack for each RS); the
   tail overlaps RS(3) with solve_v(2).
"""

import os
from contextlib import ExitStack

import numpy as np

import concourse.bass as bass
import concourse.tile as tile
from concourse import bacc, mybir
from concourse.bass import ds
from concourse.bass_utils import run_bass_kernel_spmd
from concourse.masks import make_identity, make_lower_triangular

B, M, N, R = 4, 4096, 4096, 32
NCORES = 8
MS = M // NCORES          # 512 rows per core per batch
MC = MS // 128            # 4 m-chunks of 128
NG = N // 512             # 8 n-groups of 512
NSG = 4                   # super-groups of 1024 cols (b0 load granularity)
NCH = N // 128            # 32 n-chunks of 128
EPS = 1e-8
F32 = mybir.dt.float32
F32R = mybir.dt.float32r
BF16 = mybir.dt.bfloat16
ALU = mybir.AluOpType
AX = mybir.AxisListType

_CACHE = {}
LAST_RESULT = None


def _gram_prep(nc, smp, pwp, punp, consts, b_sb, tg):
    """From Gram b precompute nbsl and ghat = (M^{-1}D'^{-1})^T.

    M^T = D' + bsl (b symmetric), W = D'^{-1} bsl strictly lower,
    (I+W)^{-1} = (I-W)(I+W^2)(I+W^4)(I+W^8)(I+W^16) exactly (W^32 = 0).
    ghat = D'^{-1} G with G = [(I+W)^{-1}]^T; then
    u_new^T = ghat.T @ (a^T - bsl^T u^T + eps).
    """
    ident32_r, masksl_r, eye_r = consts

    bd = smp.tile([R, R], F32R, tag=f"bd{tg}", name="bd", bufs=1)
    nc.vector.tensor_tensor(out=bd[:], in0=b_sb, in1=eye_r, op=ALU.mult)
    d_p = smp.tile([R, 1], F32, tag=f"dp{tg}", name="d_p", bufs=1)
    inv_p = smp.tile([R, 1], F32, tag=f"ip{tg}", name="inv_p", bufs=1)
    nc.vector.tensor_reduce(d_p[:], bd[:], axis=AX.X, op=ALU.add)
    nc.vector.tensor_scalar_add(inv_p[:], d_p[:], EPS)
    nc.vector.reciprocal(inv_p[:], inv_p[:])
    invb = bass.AP(inv_p[:].tensor, inv_p[:].offset, [inv_p[:].ap[0], [0, R]])

    bsl = smp.tile([R, R], F32R, tag=f"bsl{tg}", name="bsl", bufs=1)
    nc.vector.tensor_tensor(out=bsl[:], in0=b_sb, in1=masksl_r,
                            op=ALU.mult)
    nbsl = smp.tile([R, R], F32R, tag=f"nbsl{tg}", name="nbsl", bufs=1)
    nc.vector.tensor_scalar_mul(nbsl[:], bsl[:], -1.0)
    vw = smp.tile([R, R], F32R, tag=f"vw{tg}", name="vw", bufs=1)  # W
    nc.vector.tensor_tensor(out=vw[:], in0=bsl[:], in1=invb, op=ALU.mult)

    # transpose W -> W^T
    pwt = punp.tile([128, MC, R], F32R, tag="pun", name="pwt")
    nc.tensor.transpose(pwt[:R, 0, :], vw[:], ident32_r)
    w1 = smp.tile([R, R], F32R, tag=f"w1{tg}", name="w1", bufs=1)
    nc.scalar.copy(w1[:], pwt[:R, 0, :])

    def _mm_small(lhsT, rhs, tagn):
        p = pwp.tile([R, R], F32, tag="pw", name="pmm")
        nc.tensor.matmul(p[:], lhsT=lhsT[:], rhs=rhs[:], start=True,
                         stop=True)
        s = smp.tile([R, R], F32R, tag=f"{tagn}{tg}", name=tagn, bufs=1)
        nc.scalar.copy(s[:], p[:])
        return s

    # powers of W: plain tiles are (W^k)^T, "t" tiles are W^k
    w2 = _mm_small(vw, w1, "w2")      # (W^2)^T
    w2t = _mm_small(w1, vw, "w2t")    # W^2
    w4 = _mm_small(w2t, w2, "w4")     # (W^4)^T
    w4t = _mm_small(w2, w2t, "w4t")   # W^4
    w8 = _mm_small(w4t, w4, "w8")     # (W^8)^T
    w8t = _mm_small(w4, w4t, "w8t")   # W^8
    w16t = _mm_small(w8, w8t, "w16t")  # W^16

    # G chain: G0 = I - W^T; G <- G + (W^{2^k})^T G  (lhsT = W^{2^k})
    g = smp.tile([R, R], F32R, tag=f"g0{tg}", name="g0", bufs=1)
    nc.vector.tensor_tensor(out=g[:], in0=ident32_r, in1=w1[:],
                            op=ALU.subtract)
    for k, wkt in enumerate((w2t, w4t, w8t, w16t)):
        pg = pwp.tile([R, R], F32, tag="pw", name="pg")
        nc.tensor.matmul(pg[:], lhsT=wkt[:], rhs=g[:], start=True, stop=True)
        gn = smp.tile([R, R], F32R, tag=f"g{k + 1}{tg}", name="gn", bufs=1)
        nc.vector.tensor_tensor(out=gn[:], in0=g[:], in1=pg[:], op=ALU.add)
        g = gn
    ghat = smp.tile([R, R], F32R, tag=f"gh{tg}", name="ghat", bufs=1)
    nc.vector.tensor_tensor(out=ghat[:], in0=g[:], in1=invb, op=ALU.mult)

    return {"nbsl": nbsl, "ghat": ghat}


def _build():
    nc = bacc.Bacc("TRN2", target_bir_lowering=False, debug=False,
                   num_devices=NCORES)

    x_my = nc.dram_tensor("x_my", [B, MS, N], F32, kind="ExternalInput").ap()
    u_my = nc.dram_tensor("u_my", [B, MS, R], F32, kind="ExternalInput").ap()
    v_full = nc.dram_tensor("v_full", [B, N, R], F32,
                            kind="ExternalInput").ap()
    v_my = nc.dram_tensor("v_my", [B, MS, R], F32, kind="ExternalInput").ap()
    u_out = nc.dram_tensor("u_out", [B, MS, R], F32,
                           kind="ExternalOutput").ap()
    v_out = nc.dram_tensor("v_out", [B, MS, R], F32,
                           kind="ExternalOutput").ap()

    # f32 collective payloads; batch 3's RS is split into two halves:
    # half a = a2T cols 0:256 + b2, half b = a2T cols 256:512.
    rs_ins = [nc.dram_tensor(f"rs_in_{b}", [NCORES * R, 512 + R], F32)
              for b in range(B - 1)]
    rs_outs = [nc.dram_tensor(f"rs_out_{b}", [R, 512 + R], F32)
               for b in range(B - 1)]
    rs_in_3a = nc.dram_tensor("rs_in_3a", [NCORES * R, 256 + R], F32)
    rs_out_3a = nc.dram_tensor("rs_out_3a", [R, 256 + R], F32)
    rs_in_3b = nc.dram_tensor("rs_in_3b", [NCORES * R, 256], F32)
    rs_out_3b = nc.dram_tensor("rs_out_3b", [R, 256], F32)

    with tile.TileContext(nc) as tc, ExitStack() as ctx:
        const = ctx.enter_context(tc.tile_pool(name="const", bufs=1))
        xbp = ctx.enter_context(tc.tile_pool(name="xbp", bufs=1))
        xtp = ctx.enter_context(tc.tile_pool(name="xtp", bufs=8))
        vp = ctx.enter_context(tc.tile_pool(name="vp", bufs=2))
        smp = ctx.enter_context(tc.tile_pool(name="smp", bufs=2))
        a2sp = ctx.enter_context(tc.tile_pool(name="a2sp", bufs=1))
        # PSUM banks: ppt 2 + pa1 1 + pa2 1 + pw 1 + pun 1 + psol 2 = 8
        ppt = ctx.enter_context(tc.tile_pool(name="ppt", bufs=2,
                                             space="PSUM"))
        pa1p = ctx.enter_context(tc.tile_pool(name="pa1", bufs=1,
                                              space="PSUM"))
        pa2p = ctx.enter_context(tc.tile_pool(name="pa2", bufs=1,
                                              space="PSUM"))
        pwp = ctx.enter_context(tc.tile_pool(name="pw", bufs=1,
                                             space="PSUM"))
        punp = ctx.enter_context(tc.tile_pool(name="pun", bufs=1,
                                              space="PSUM"))
        psolp = ctx.enter_context(tc.tile_pool(name="psol", bufs=2,
                                               space="PSUM"))

        ident128_f = const.tile([128, 128], F32)
        make_identity(nc, ident128_f)
        ident128_r = const.tile([128, 128], F32R)
        nc.vector.tensor_copy(ident128_r[:], ident128_f[:])
        ident32_f = const.tile([R, R], F32)
        make_identity(nc, ident32_f)
        masksl_f = const.tile([R, R], F32)
        make_lower_triangular(nc, masksl_f, val=1.0, diag=False)
        ident32_r = const.tile([R, R], F32R)
        nc.vector.tensor_copy(ident32_r[:], ident32_f[:])
        masksl_r = const.tile([R, R], F32R)
        nc.vector.tensor_copy(masksl_r[:], masksl_f[:])
        consts = (ident32_r[:], masksl_r[:], ident32_r[:])

        xbs = {}

        def xb_tile(b):
            t = xbp.tile([128, MC, N], F32R, tag=f"xb{b % 2}", name="xb")
            xbs[b] = t
            return t

        def x_dram(b):
            return x_my[b].rearrange("(i p) n -> p i n", p=128).bitcast(F32R)

        # v_full chunk layout [128 (n%128), 32 (n//128), 32 r], loaded in
        # 4 pieces of 8 chunks (1024 descriptors each), queues alternated.
        vts = {}

        def v_dram(b):
            return (v_full[b].rearrange("(c p) r -> p c r", p=128)
                    .bitcast(F32R))

        def load_v_piece(b, piece, eng):
            if b not in vts:
                vts[b] = vp.tile([128, NCH, R], F32R, tag=f"v32{b}",
                                 name="v32", bufs=1)
            c0 = piece * 8
            eng.dma_start(vts[b][:, c0:c0 + 8, :],
                          v_dram(b)[:, c0:c0 + 8, :])

        # batch 0 startup: x sg0/sg1 on sync, v(0) pieces on scalar
        xb0 = xb_tile(0)
        nc.sync.dma_start(xb0[:, :, 0:1024], x_dram(0)[:, :, 0:1024])
        load_v_piece(0, 0, nc.scalar)
        nc.sync.dma_start(xb0[:, :, 1024:2048], x_dram(0)[:, :, 1024:2048])
        load_v_piece(0, 1, nc.scalar)

        u_preps = {}

        def emit_prep(b):
            pb1 = pwp.tile([R, R], F32, tag="pw", name="pb1")
            for j in range(NCH):
                nc.tensor.matmul(pb1[:], lhsT=vts[b][:, j, :],
                                 rhs=vts[b][:, j, :], start=(j == 0),
                                 stop=(j == NCH - 1), skip_group_check=True)
            b1_sb = smp.tile([R, R], F32R, tag=f"b1s{b}", name="b1_sb",
                             bufs=1)
            nc.scalar.copy(b1_sb[:], pb1[:])
            u_preps[b] = _gram_prep(nc, smp, pwp, punp, consts, b1_sb[:],
                                    f"u{b}")

        state = {}

        def back_transpose(zout, cols, out32, tg):
            """zout [R, len(cols)*128] (SBUF) -> out32 chunks via PE."""
            pun = punp.tile([128, MC, R], F32R, tag="pun", name=f"pun{tg}")
            for k, i in enumerate(cols):
                nc.tensor.transpose(pun[:, i, :],
                                    zout[:, k * 128:(k + 1) * 128],
                                    ident32_r[:])
            for k, i in enumerate(cols):
                nc.scalar.copy(out32[:, i, :], pun[:, i, :])

        def usolve_finish(b):
            """Epilogue of batch b's u-solve (pa1 group already closed)."""
            pa1 = state[b]["pa1"]
            zin = smp.tile([R, MS], F32R, tag="zinu", name="zinu", bufs=1)
            nc.vector.tensor_scalar_add(zin[:], pa1[:], EPS)
            pzu = psolp.tile([R, MS], F32, tag="psol", name="pzu")
            nc.tensor.matmul(pzu[:], lhsT=u_preps[b]["ghat"][:], rhs=zin[:],
                             start=True, stop=True, skip_group_check=True)
            zout = smp.tile([R, MS], F32R, tag="zou", name="zou", bufs=1)
            nc.scalar.copy(zout[:], pzu[:])
            un32 = smp.tile([128, MC, R], F32R, tag="un32", name="un32",
                            bufs=1)
            back_transpose(zout, range(MC), un32, "u")
            nc.scalar.dma_start(
                u_out[b].rearrange("(i p) r -> p i r", p=128).bitcast(F32R),
                un32[:])
            state[b]["un32"] = un32

        def emit_phase2_part(bp, gp):
            un_p = state[bp]["un32"]
            xb_p = xbs[bp]
            pa2 = pa2p.tile([R, MS], F32, tag="pa2", name="pa2")
            for i in range(MC):
                nc.tensor.matmul(
                    pa2[:], lhsT=un_p[:, i, :],
                    rhs=xb_p[:, i, gp * 512:(gp + 1) * 512],
                    start=(i == 0), stop=(i == MC - 1),
                    skip_group_check=True)
            a2st = a2sp.tile([R, MS], F32, tag="a2st", name="a2st")
            nc.vector.tensor_copy(a2st[:], pa2[:])
            if bp < B - 1:
                nc.scalar.dma_start(
                    rs_ins[bp].ap()[ds(gp * R, R), 0:512], a2st[:])
            else:
                nc.scalar.dma_start(
                    rs_in_3a.ap()[ds(gp * R, R), 0:256], a2st[:, 0:256])
                nc.scalar.dma_start(
                    rs_in_3b.ap()[ds(gp * R, R), 0:256], a2st[:, 256:512])

        def bcast_b2(b2st, rs_dram, col0):
            # one DMA writing all 8 replicas of b2: out iterates (r, c, k)
            out_ap = rs_dram.ap()[:, col0:col0 + R].rearrange(
                "(c r) k -> r c k", c=NCORES)
            src = b2st[:]
            in_ap = bass.AP(src.tensor, src.offset,
                            [src.ap[0], [0, NCORES], src.ap[1]])
            nc.scalar.dma_start(out_ap, in_ap)

        def emit_b2(bp):
            un_p = state[bp]["un32"]
            pb2 = pwp.tile([R, R], F32, tag="pw", name="pb2")
            for i in range(MC):
                nc.tensor.matmul(pb2[:], lhsT=un_p[:, i, :],
                                 rhs=un_p[:, i, :], start=(i == 0),
                                 stop=(i == MC - 1), skip_group_check=True)
            b2st = a2sp.tile([R, R], F32, tag="b2st", name="b2st")
            nc.scalar.copy(b2st[:], pb2[:])
            if bp < B - 1:
                bcast_b2(b2st, rs_ins[bp], 512)
            else:
                bcast_b2(b2st, rs_in_3a, 256)

        def emit_rs(bp):
            nc.gpsimd.collective_compute(
                "ReduceScatter", ALU.add,
                replica_groups=[list(range(NCORES))],
                ins=[rs_ins[bp].ap()], outs=[rs_outs[bp].ap()])

        def load_rs_result(bp):
            # gpsimd queue: serialized behind the collective itself
            st = state[bp]
            if bp < B - 1:
                a2t = smp.tile([R, 512 + R], F32, tag="a2t", name="a2t",
                               bufs=2)
                nc.gpsimd.dma_start(a2t[:], rs_outs[bp].ap())
                st["a2t_halves"] = [(a2t[:, 0:256], 0, 256),
                                    (a2t[:, 256:512], 256, 512)]
                st["b2"] = a2t[:, 512:512 + R]
            else:
                a2ta = smp.tile([R, 256 + R], F32, tag="a2t", name="a2ta",
                                bufs=2)
                nc.gpsimd.dma_start(a2ta[:], rs_out_3a.ap())
                st["a2t_halves"] = [(a2ta[:, 0:256], 0, 256), None]
                st["b2"] = a2ta[:, 256:256 + R]

        def load_rs_result_3b(bp):
            st = state[bp]
            a2tb = smp.tile([R, 256], F32, tag="a2tb", name="a2tb", bufs=1)
            nc.gpsimd.dma_start(a2tb[:], rs_out_3b.ap())
            st["a2t_halves"][1] = (a2tb[:], 256, 512)

        def emit_prep_v(b):
            st = state[b]
            st["vprep"] = _gram_prep(nc, smp, pwp, punp, consts, st["b2"],
                                     f"v{b}")

        def solve_v_ps(b):
            st = state[b]
            ps = psolp.tile([R, MS], F32, tag="psol", name="ps")
            nc.tensor.matmul(ps[:], lhsT=st["vprep"]["nbsl"][:],
                             rhs=st["vT"][:], start=True, stop=True,
                             skip_group_check=True)
            st["ps"] = ps

        def solve_v_zin(b, h):
            st = state[b]
            a2t, c0, c1 = st["a2t_halves"][h]
            zin = smp.tile([R, 256], F32R, tag=f"zinv{h}", name="zin",
                           bufs=1)
            nc.vector.scalar_tensor_tensor(
                out=zin[:], in0=st["ps"][:, c0:c1], scalar=EPS,
                in1=a2t, op0=ALU.add, op1=ALU.add)
            st[f"zin{h}"] = zin

        def solve_v_fin(b, h, vn32):
            st = state[b]
            pz = psolp.tile([R, 256], F32, tag="psol", name=f"pz{h}")
            nc.tensor.matmul(pz[:], lhsT=st["vprep"]["ghat"][:],
                             rhs=st[f"zin{h}"][:], start=True, stop=True,
                             skip_group_check=True)
            zov = smp.tile([R, 256], F32R, tag=f"zov{h}", name="zov",
                           bufs=1)
            nc.scalar.copy(zov[:], pz[:])
            back_transpose(zov, (2 * h, 2 * h + 1), vn32, f"v{h}")

        def vn_tile():
            return smp.tile([128, MC, R], F32R, tag="vn32", name="vn32",
                            bufs=1)

        def vout_dma(b, vn32):
            nc.scalar.dma_start(
                v_out[b].rearrange("(i p) r -> p i r", p=128).bitcast(F32R),
                vn32[:])

        def solve_v_full(b):
            solve_v_ps(b)
            vn32 = vn_tile()
            solve_v_zin(b, 0)
            solve_v_fin(b, 0, vn32)
            solve_v_zin(b, 1)
            solve_v_fin(b, 1, vn32)
            vout_dma(b, vn32)

        # ================= main batch loop =================
        for b in range(B):
            xb = xbs[b]

            u32 = vp.tile([128, MC, R], F32R, tag="u32", name="u32")
            nc.scalar.dma_start(
                u32[:],
                u_my[b].rearrange("(i p) r -> p i r", p=128).bitcast(F32R))
            vm32 = vp.tile([128, MC, R], F32R, tag="vm32", name="vm32")
            nc.scalar.dma_start(
                vm32[:],
                v_my[b].rearrange("(i p) r -> p i r", p=128).bitcast(F32R))
            uvT = {}

            def emit_uvT():
                put = psolp.tile([R, MS], F32R, tag="psol", name="put")
                for i in range(MC):
                    nc.tensor.transpose(put[:, i * 128:(i + 1) * 128],
                                        u32[:, i, :], ident128_r[:])
                uT = smp.tile([R, MS], F32R, tag="uT", name="uT", bufs=1)
                nc.scalar.copy(uT[:], put[:])
                pvt = psolp.tile([R, MS], F32R, tag="psol", name="pvt")
                for i in range(MC):
                    nc.tensor.transpose(pvt[:, i * 128:(i + 1) * 128],
                                        vm32[:, i, :], ident128_r[:])
                vT = smp.tile([R, MS], F32R, tag="vT", name="vT", bufs=3)
                nc.scalar.copy(vT[:], pvt[:])
                uvT["uT"], uvT["vT"] = uT, vT

            # ---------------- phase 1: stream x ----------------
            pa1 = pa1p.tile([R, MS], F32, tag="pa1", name="pa1")

            def emit_a1(q):
                for j2 in range(4):
                    j = 4 * q + j2
                    xt = p1xt[j]
                    nc.tensor.matmul(pa1[:], lhsT=vts[b][:, j, :],
                                     rhs=xt.rearrange("p a b -> p (a b)"),
                                     start=(j == 0), stop=False,
                                     skip_group_check=True)

            p1xt = {}
            for q in range(NG):          # quartet q covers j = 4q..4q+3
                if b == 0 and q % 2 == 0 and q // 2 + 2 < NSG:
                    sg_n = q // 2 + 2
                    nc.sync.dma_start(
                        xb[:, :, sg_n * 1024:(sg_n + 1) * 1024],
                        x_dram(0)[:, :, sg_n * 1024:(sg_n + 1) * 1024])
                for j2 in range(4):
                    j = 4 * q + j2
                    pt = ppt.tile([128, MC, 128], F32R, tag="pt", name="pt")
                    for i in range(MC):
                        nc.tensor.transpose(
                            pt[:, i], xb[:, i, j * 128:(j + 1) * 128],
                            ident128_r[:])
                    xt = xtp.tile([128, MC, 128], F32R, tag="xt", name="xt")
                    nc.vector.tensor_copy(xt[:], pt[:])
                    p1xt[j] = xt
                if q >= 1:
                    emit_a1(q - 1)
                # slot work overlapped with the stream
                if b >= 1:
                    if q == 0:
                        usolve_finish(b - 1)
                        for gp in range(4):
                            emit_phase2_part(b - 1, gp)
                    elif q == 1:
                        for gp in range(4, 8):
                            emit_phase2_part(b - 1, gp)
                        emit_b2(b - 1)
                        emit_rs(b - 1)
                    elif q == 6:
                        load_rs_result(b - 1)
                if b >= 2 and q == 5:
                    emit_prep_v(b - 2)
                if q == 2:
                    emit_uvT()
                    if b == 0:
                        load_v_piece(0, 2, nc.sync)
                        load_v_piece(0, 3, nc.scalar)
                if q == (5 if b == 0 else 3):
                    emit_prep(b)
                if q == 4 and b + 1 < B:
                    xb_n = xb_tile(b + 1)
                    nc.sync.dma_start(xb_n[:, :, 0:1024],
                                      x_dram(b + 1)[:, :, 0:1024])
                    nc.sync.dma_start(xb_n[:, :, 1024:4096],
                                      x_dram(b + 1)[:, :, 1024:4096])
                if q in (5, 6) and b + 1 < B:
                    load_v_piece(b + 1, 2 * (q - 5), nc.sync)
                    load_v_piece(b + 1, 2 * (q - 5) + 1, nc.scalar)

            # ------------- stream end: a1 flush + v-solve(b-2) ----------
            uT, vT = uvT["uT"], uvT["vT"]
            if b >= 2:
                solve_v_ps(b - 2)
            emit_a1(NG - 1)
            if b >= 2:
                solve_v_zin(b - 2, 0)
                solve_v_zin(b - 2, 1)
            nc.tensor.matmul(pa1[:], lhsT=u_preps[b]["nbsl"][:], rhs=uT[:],
                             start=False, stop=True, skip_group_check=True)
            if b >= 2:
                vn32 = vn_tile()
                solve_v_fin(b - 2, 0, vn32)
                solve_v_fin(b - 2, 1, vn32)
                vout_dma(b - 2, vn32)

            state[b] = {"pa1": pa1, "uT": uT, "vT": vT}

        # ================= tail: batches 2 and 3 =================
        bl = B - 1
        usolve_finish(bl)
        emit_b2(bl)
        for gp in range(8):
            emit_phase2_part(bl, gp)
        nc.gpsimd.collective_compute(
            "ReduceScatter", ALU.add,
            replica_groups=[list(range(NCORES))],
            ins=[rs_in_3a.ap()], outs=[rs_out_3a.ap()])
        load_rs_result(bl)
        nc.gpsimd.collective_compute(
            "ReduceScatter", ALU.add,
            replica_groups=[list(range(NCORES))],
            ins=[rs_in_3b.ap()], outs=[rs_out_3b.ap()])
        load_rs_result_3b(bl)
        # solve_v(2) overlaps RS(3a/3b)
        emit_prep_v(bl - 1)
        solve_v_full(bl - 1)
        emit_prep_v(bl)
        solve_v_full(bl)

    nc.compile()
    return nc


def kernel(x, u, v):
    global LAST_RESULT
    if "nc" not in _CACHE:
        _CACHE["nc"] = _build()
    nc = _CACHE["nc"]

    x = np.ascontiguousarray(x, dtype=np.float32)
    u = np.ascontiguousarray(u, dtype=np.float32)
    v = np.ascontiguousarray(v, dtype=np.float32)

    in_maps = []
    for c in range(NCORES):
        sl = slice(c * MS, (c + 1) * MS)
        in_maps.append({
            "x_my": np.ascontiguousarray(x[:, sl, :]),
            "u_my": np.ascontiguousarray(u[:, sl, :]),
            "v_full": v,
            "v_my": np.ascontiguousarray(v[:, sl, :]),
        })

    res = run_bass_kernel_spmd(nc, in_maps, list(range(NCORES)),
                               trace=os.environ.get("KBENCH_TRACE") == "1")
    LAST_RESULT = res
    u_new = np.concatenate([res.results[c]["u_out"] for c in range(NCORES)],
                           axis=1)
    v_new = np.concatenate([res.results[c]["v_out"] for c in range(NCORES)],
                           axis=1)
    return (u_new, v_new)
